# revision 68
# baseline (speedup 1.0000x reference)
"""Trainium2 Bass kernel: AnisotropicHomogeneousNN (raw-bass, manual sems).

Math per sample: solve sum_i e^{-2 r_i s} x_i^2 = 1 by log-Newton
(1 algebraic iter at s=0 + NITER_FULL full iters, bf16 noise floor),
then out = MLP(x * e^{-r s}) * e^{s}.

Distribution: pure data parallelism, batch split over 8 cores.

On-chip layout: feature-major "2-stacked" bf16 tiles [128, 512]:
partition p<64 = feature p of even sample of a pair, p>=64 = feature
p-64 of odd sample; columns = pair index.  PE does transposes in/out
(identity matmul), per-sample reductions (F, G) via per-tile one-hot
lhsT accumulated into one [128,512] PSUM, the s broadcast (bf16 hi/lo
split), and the block-diagonal MLP.  ACT: exp/ln/relu/copies with
per-partition scale/bias.  DVE: elementwise mults + Newton updates.

Raw Block style: this toolchain's walrus rejects instructions with
more than ~1 embedded sync wait, so Tile-generated scheduling cannot
compile; all waits here are standalone wait_ge instructions.
"""

import numpy as np
import ml_dtypes

import concourse.bass as bass
import concourse.mybir as mybir
from concourse.bass_utils import run_bass_kernel_spmd

B, N, H, O = 262144, 64, 256, 64
NCORES = 8
BC = B // NCORES      # samples per core      32768
BP = BC // 2          # sample pairs per core 16384
T = 512               # pair-columns per tile
NT = BP // T          # tiles: 32
NBLK = BP // 128      # [128,128] transpose blocks: 128
NITER_FULL = 1
NU = 1.0

f32 = mybir.dt.float32
bf16 = mybir.dt.bfloat16
AF = mybir.ActivationFunctionType
ALU = mybir.AluOpType

_last_exec_ns = None


def _host_consts(r, W1, b1, W2, b2):
    r = np.asarray(r, np.float32)
    W1 = np.asarray(W1, np.float32)
    b1 = np.asarray(b1, np.float32)
    W2 = np.asarray(W2, np.float32)
    b2 = np.asarray(b2, np.float32)

    # reduce lhsT [128, NT*128]: col = 128*t + m.  Rows of FG psum:
    # F_e -> m=t, F_o -> m=32+t, G_e -> m=64+t, G_o -> m=96+t
    RED = np.zeros((128, NT * 128), np.float32)
    for t in range(NT):
        RED[0:64, 128 * t + t] = 1.0
        RED[64:128, 128 * t + 32 + t] = 1.0
        RED[0:64, 128 * t + 64 + t] = 2.0 * r
        RED[64:128, 128 * t + 96 + t] = 2.0 * r

    # broadcast lhsT: per tile t, out row p<64 sums SHL rows {t, 64+t}
    # (hi_e+lo_e), p>=64 sums rows {32+t, 96+t} (hi_o+lo_o)
    BCT = np.zeros((128, NT * 128), np.float32)
    for t in range(NT):
        BCT[t, 128 * t + 0:128 * t + 64] = 1.0
        BCT[64 + t, 128 * t + 0:128 * t + 64] = 1.0
        BCT[32 + t, 128 * t + 64:128 * t + 128] = 1.0
        BCT[96 + t, 128 * t + 64:128 * t + 128] = 1.0

    W1BD = np.zeros((128, 4 * 128), np.float32)
    W2BD = np.zeros((128, 4 * 128), np.float32)
    for c in range(4):
        W1BD[0:64, 128 * c + 0:128 * c + 64] = W1[64 * c:64 * c + 64, :].T
        W1BD[64:128, 128 * c + 64:128 * c + 128] = W1[64 * c:64 * c + 64, :].T
        W2BD[0:64, 128 * c + 0:128 * c + 64] = W2[:, 64 * c:64 * c + 64].T
        W2BD[64:128, 128 * c + 64:128 * c + 128] = W2[:, 64 * c:64 * c + 64].T

    B1BD = np.zeros((128, 4), np.float32)
    for c in range(4):
        B1BD[0:64, c] = b1[64 * c:64 * c + 64]
        B1BD[64:128, c] = b1[64 * c:64 * c + 64]
    B2BD = np.zeros((128, 1), np.float32)
    B2BD[0:64, 0] = b2
    B2BD[64:128, 0] = b2
    B2R = B2BD.T.copy()

    RNEG2 = np.zeros((128, 1), np.float32)
    RNEG2[0:64, 0] = -2.0 * r
    RNEG2[64:128, 0] = -2.0 * r
    RNEG1 = 0.5 * RNEG2
    IDB = np.eye(128, dtype=np.float32)

    # Gauss-2 fixed nodes of the (unweighted) r distribution: the init
    # model F(s) ~ w1 e^{-2 l1 s} + w2 e^{-2 l2 s} with per-sample weights
    # from (m0, m1) = (F, G/2) at s=0.
    rd = r.astype(np.float64)
    mu1, mu2, mu3 = rd.mean(), (rd**2).mean(), (rd**3).mean()
    det = mu1 * mu1 - mu2
    pp = (mu1 * mu2 - mu3) / det
    qq = (mu1 * mu3 - mu2 * mu2) / det
    sq = np.sqrt(max(pp * pp + 4 * qq, 1e-12))
    l1 = (pp + sq) / 2.0
    l2 = (pp - sq) / 2.0
    ic = 1.0 / (l1 - l2)
    c3 = 1.0 / (2.0 * l2)
    CIN = np.zeros((64, 8), np.float32)
    CIN[:, 0] = l1
    CIN[:, 1] = l2
    CIN[:, 2] = ic
    CIN[:, 3] = c3
    CIN[:, 4] = np.log(ic) * c3
    CIN[:, 5] = -2.0 * l1
    CIN[:, 6] = -2.0 * l2

    tobf = lambda a: a.astype(ml_dtypes.bfloat16)
    return {
        "RED": tobf(RED), "BCT": tobf(BCT), "IDB": tobf(IDB),
        "W1BD": tobf(W1BD), "W2BD": tobf(W2BD),
        "B1BD": B1BD, "B2BD": B2BD, "B2R": tobf(B2R), "ONESC": np.ones((1, 512), ml_dtypes.bfloat16), "RNEG2": RNEG2, "RNEG1": RNEG1,
        "CIN": CIN,
    }


def _build():
    from contextlib import ExitStack
    nc = bass.Bass()

    x = nc.declare_dram_parameter("x", [BC, N], f32, isOutput=False)
    RED = nc.declare_dram_parameter("RED", [128, NT * 128], bf16, isOutput=False)
    BCT = nc.declare_dram_parameter("BCT", [128, NT * 128], bf16, isOutput=False)
    IDB = nc.declare_dram_parameter("IDB", [128, 128], bf16, isOutput=False)
    W1BD = nc.declare_dram_parameter("W1BD", [128, 512], bf16, isOutput=False)
    W2BD = nc.declare_dram_parameter("W2BD", [128, 512], bf16, isOutput=False)
    B1BD = nc.declare_dram_parameter("B1BD", [128, 4], f32, isOutput=False)
    B2BD = nc.declare_dram_parameter("B2BD", [128, 1], f32, isOutput=False)
    B2R = nc.declare_dram_parameter("B2R", [1, 128], bf16, isOutput=False)
    ONESC = nc.declare_dram_parameter("ONESC", [1, 512], bf16, isOutput=False)
    RNEG2 = nc.declare_dram_parameter("RNEG2", [128, 1], f32, isOutput=False)
    RNEG1 = nc.declare_dram_parameter("RNEG1", [128, 1], f32, isOutput=False)
    CIN = nc.declare_dram_parameter("CIN", [64, 8], f32, isOutput=False)
    out = nc.declare_dram_parameter("out", [BC, N], f32, isOutput=True)

    xv = x.rearrange("(b p two) f -> b p (two f)", p=128, two=2)
    # tile-granular load view: [32, 128, 4, 128] f32, 512B contiguous chunks
    xvt = x.rearrange("(t b p two) f -> t p b (two f)", b=4, p=128, two=2)
    # 4-block store view: tile t covers out rows [1024t, 1024(t+1))
    ov4 = out.rearrange("(t beta p e) f -> t p beta (e f)", beta=4, p=128, e=2)

    NP = NT // 2          # pairs of tiles: 16
    NCONST = 12

    es = ExitStack()
    with es:
        _n = [0]
        def sbuf(shape, dt):
            _n[0] += 1
            return es.enter_context(nc.sbuf_tensor(f"sb{_n[0]}", shape, dt))
        def psum(shape, dt):
            _n[0] += 1
            return es.enter_context(nc.psum_tensor(f"ps{_n[0]}", shape, dt))
        sem = lambda name: es.enter_context(nc.semaphore(name))

        XT = sbuf([128, BP], bf16)
        A = sbuf([128, BP], bf16)
        red = sbuf([128, NT * 128], bf16)
        bct = sbuf([128, NT * 128], bf16)
        idb = sbuf([128, 128], bf16)
        w1 = sbuf([128, 512], bf16)
        w2 = sbuf([128, 512], bf16)
        b1t = sbuf([128, 4], f32)
        b2t = sbuf([128, 1], f32)
        b2r = sbuf([1, 128], bf16)
        ones = sbuf([1, T], bf16)   # loaded from ONESC
        rn2 = sbuf([128, 1], f32)
        rn1 = sbuf([128, 1], f32)
        cin = sbuf([64, 8], f32)
        # fixed-node 2-exponential init scratch ([64, T] f32 each)
        T1 = sbuf([64, T], f32)
        G2 = sbuf([64, T], f32)
        W2N = sbuf([64, T], f32)
        S0 = sbuf([64, T], f32)
        W2V = sbuf([64, T], f32)
        W1V = sbuf([64, T], f32)
        T1M = sbuf([64, T], f32)
        T2M = sbuf([64, T], f32)
        FM = sbuf([64, T], f32)
        GM = sbuf([64, T], f32)
        E1M = sbuf([64, T], f32)
        E2M = sbuf([64, T], f32)
        LW = sbuf([64, T], f32)
        S = sbuf([64, T], f32)
        SHL = sbuf([128, T], bf16)
        LF = sbuf([64, T], f32)
        RG = sbuf([64, T], f32)
        P1 = sbuf([64, T], f32)
        XF = [sbuf([128, 512], f32) for _ in range(4)]
        XB = [sbuf([128, 512], bf16) for _ in range(4)]
        EM = [sbuf([128, 2 * T], bf16) for _ in range(3)]
        MM = [sbuf([128, 2 * T], bf16) for _ in range(3)]
        D = [sbuf([128, 2 * T], bf16) for _ in range(2)]
        ES = [sbuf([128, 2 * T], f32) for _ in range(2)]
        XS = [sbuf([128, 2 * T], bf16) for _ in range(2)]
        HR = [[sbuf([128, T], bf16) for _ in range(4)] for _ in range(2)]
        Q = [sbuf([128, 2 * T], f32) for _ in range(2)]
        OF = [sbuf([128, 2 * T], bf16) for _ in range(2)]
        OC = [sbuf([128, 2 * T], f32) for _ in range(2)]

        # PSUM (8 banks): PB 2 (FG in Newton / OP pair in final),
        # SBP2 2x2, TPI 1, HP 1
        PB = psum([128, 2 * T], f32)
        SBP = [psum([128, 2 * T], f32) for _ in range(2)]
        TPI = psum([128, 2048], bf16)   # two [128,1024] halves
        FG = PB[:, 0:T]
        # 3-deep Newton bcast ring: SBP[0], SBP[1], and the TPI banks
        # (idle between the load phase and the final phase) bitcast to f32
        TPIF = TPI[:, 0:2048].bitcast(f32)   # [128, 1024] f32 view
        SBPR = [SBP[0][:], SBP[1][:], TPIF]
        # final phase: HP chunks alias SBP[1]'s two banks (Newton is done
        # with SBP by then except as re-gated by sems)
        HP = [SBP[1][:, 0:T], SBP[1][:, T:2 * T]]

        s_cdma = sem("s_cdma")
        s_cdma1 = sem("s_cdma1")  # RED/BCT/IDB only (gates transposes)
        s_ldt = [sem(f"s_ldt{i}") for i in range(4)]
        s_st2 = [sem(f"s_st{i}") for i in range(2)]
        s_pool = sem("s_pool")
        s_pti = sem("s_pti")      # PE input transposes (per block)
        s_xt = sem("s_xt")        # ACT XT copies (per 8-block group)
        s_a = sem("s_a")          # DVE squares (per 8-block group)
        s_ln = sem("s_ln")
        s_iv = sem("s_iv")        # DVE init-chain progress (it=0)
        s_sml = sem("s_sml")
        s_sb = sem("s_sb")        # PE bcast (per pair, cumulative)
        s_e = sem("s_e")          # ACT exp (per pair, cumulative)
        s_m = sem("s_m")          # DVE m (per pair, cumulative)
        s_red = sem("s_red")      # PE reduce mms (per mm, cumulative)
        s_des = sem("s_des")      # ACT d+es (per pair)
        s_d = sem("s_d")          # ACT d alone (per pair, gates xs)
        s_xs = sem("s_xs")        # DVE xs (per pair)
        s_h = sem("s_h")          # PE mm1 (per mm: 8/pair)
        s_ra = sem("s_ra")        # ACT relus (2/tile)
        s_rv = sem("s_rv")        # DVE relu c=2 (1/tile)
        s_rp = sem("s_rp")        # Pool relu c=3 (1/tile)
        s_op = sem("s_op")        # PE mm2 half done (per tile)
        s_q = sem("s_q")          # ACT q (per pair)
        s_of = sem("s_of")        # DVE of (per pair)
        s_pto = sem("s_pto")      # PE out transposes (per pair)
        s_oc = sem("s_oc")        # ACT oc (per pair)

        CD = (NCONST - 3) * 16    # consts on s_cdma (3 ride s_cdma1)
        NG = NBLK // 8            # 8-block groups: 16

        with nc.Block() as block:

            @block.sync
            def _(eng):
                # tile loads: [128, 512] f32 each
                for t in range(NT):
                    if t >= 4:
                        eng.wait_ge(s_pool, t - 3)
                    eng.dma_start(
                        out=XF[t % 4][:].rearrange("p (b q) -> p b q", b=4),
                        in_=xvt[t]).then_inc(s_ldt[t % 4], 16)
                # stores: 2 per pair of tiles, [128,512] f32 each
                for u in range(NP):
                    eng.wait_ge(s_oc, u + 1)
                    for h in range(2):
                        eng.dma_start(
                            out=ov4[2 * u + h],
                            in_=OC[u % 2][:, T * h:T * (h + 1)].rearrange(
                                "p (beta ef) -> p beta ef", beta=4)) \
                           .then_inc(s_st2[u % 2], 16)

            @block.gpsimd
            def _(eng):
                for t in range(NT):
                    eng.wait_ge(s_ldt[t % 4], 16 * (t // 4 + 1))
                    if t >= 4:
                        # XB slot reused: tile t-4's 4 transposes done
                        eng.wait_ge(s_pti, 4 * (t - 4) + 4)
                    eng.tensor_copy(XB[t % 4][:], XF[t % 4][:]) \
                       .then_inc(s_pool, 1)

            @block.tensor
            def _(eng):
                eng.wait_ge(s_cdma1, 48)   # RED/BCT/IDB (complete counter)
                # input transposes: 8-block groups into TPI (single buf)
                for g in range(NG):
                    if g >= 2:
                        eng.wait_ge(s_xt, g - 1)     # TPI half free
                    for j in range(8):
                        t = 2 * g + (j // 4)
                        eng.wait_ge(s_pool, t + 1)
                        eng.transpose(
                            TPI[:, 1024 * (g % 2) + 128 * j:
                                1024 * (g % 2) + 128 * (j + 1)],
                            XB[t % 4][:, 128 * (j % 4):128 * (j % 4 + 1)],
                            idb[:]).then_inc(s_pti, 1)
                # Newton iter 0: FG from A (2 mms per pair)
                eng.wait_ge(s_cdma, CD)   # remaining consts (w1/w2/...)
                for u in range(NP):
                    eng.wait_ge(s_a, u + 1)   # squares of group u complete
                    for h in range(2):
                        t = 2 * u + h
                        eng.matmul(FG, red[:, 128 * t:128 * (t + 1)],
                                   A[:, T * t:T * (t + 1)],
                                   start=(t == 0), stop=(t == NT - 1)) \
                           .then_inc(s_red, 1)
                # full iterations (3-deep bcast ring SBPR)
                for it in range(NITER_FULL):
                    def bcast(u, it=it):
                        if u == 0:
                            eng.wait_ge(s_sml, it + 1)
                        if u == 2:
                            eng.wait_ge(s_xt, NG)    # TPI free of load use
                        if u >= 3:
                            eng.wait_ge(s_e, NP * it + u - 2)
                        for h in range(2):
                            t = 2 * u + h
                            ins = eng.matmul(
                                SBPR[u % 3][:, T * h:T * (h + 1)],
                                bct[:, 128 * t:128 * (t + 1)], SHL[:],
                                start=True, stop=True)
                            if h == 1:
                                ins.then_inc(s_sb, 1)
                    bcast(0)
                    bcast(1)
                    bcast(2)
                    for u in range(NP):
                        eng.wait_ge(s_m, NP * it + u + 1)
                        for h in range(2):
                            t = 2 * u + h
                            eng.matmul(FG, red[:, 128 * t:128 * (t + 1)],
                                       MM[u % 3][:, T * h:T * (h + 1)],
                                       start=(t == 0), stop=(t == NT - 1)) \
                               .then_inc(s_red, 1)
                        if u + 3 < NP:
                            bcast(u + 3)
                # final phase.  bcast(u) for the NEXT pair is emitted
                # between mm1(u-1) and mm2(u-1) so the ACT d/es block of
                # pair u overlaps the mm chain of pair u-1.
                def bcast2(u):
                    if u == 0:
                        eng.wait_ge(s_sml, NITER_FULL + 1)
                    if u >= 1:
                        eng.wait_ge(s_des, u)    # SBP[0] single-buffered
                    for h in range(2):
                        t = 2 * u + h
                        ins = eng.matmul(SBP[0][:, T * h:T * (h + 1)],
                                         bct[:, 128 * t:128 * (t + 1)],
                                         SHL[:], start=True, stop=True)
                        if h == 1:
                            ins.then_inc(s_sb, 1)
                bcast2(0)
                for u in range(NP):
                    # out transposes for the previous pair (pipelined,
                    # emitted before this pair's MLP so ACT's oc() is never
                    # gated behind work that needs this pair's d/es)
                    def xpose(v):
                        eng.wait_ge(s_of, v + 1)
                        if v < 2:
                            eng.wait_ge(s_xt, NG)    # TPI free of load use
                        else:
                            eng.wait_ge(s_oc, v - 1)  # TPI half free
                        for j in range(8):
                            ins = eng.transpose(
                                TPI[:, 1024 * (v % 2) + 128 * j:
                                    1024 * (v % 2) + 128 * (j + 1)],
                                OF[v % 2][:, 128 * j:128 * (j + 1)], idb[:])
                            if j == 7:
                                ins.then_inc(s_pto, 1)
                    if u >= 1:
                        xpose(u - 1)
                    # mm1: HP single bank; cycle mm1(t,c) -> relu(t,c)
                    eng.wait_ge(s_xs, u + 1)
                    for h in range(2):
                        t = 2 * u + h
                        for c in range(4):
                            # HP[0]: mm1(t,0) relu(t,0) mm1(t,2) relu(t,2)
                            # -> mm1(t+1,0); HP[1] same with 1,3.
                            # relus: ACT c in {0,2}, DVE c in {1,3} so the
                            # two relus of an mm1 pair run on both engines.
                            if c < 2:
                                if t >= 1:
                                    sm = s_ra if c == 0 else s_rv
                                    eng.wait_ge(sm, 2 * t)
                            else:
                                sm = s_ra if c == 2 else s_rv
                                eng.wait_ge(sm, 2 * t + 1)
                            eng.matmul(HP[c % 2], w1[:, 128 * c:128 * (c + 1)],
                                       XS[u % 2][:, T * h:T * (h + 1)],
                                       start=True, stop=True).then_inc(s_h, 1)
                    # next pair's s-broadcast (frees ACT to run d/es early)
                    if u + 1 < NP:
                        bcast2(u + 1)
                    # mm2 into PB halves
                    if u == 0:
                        eng.wait_ge(s_sml, NITER_FULL + 1)  # PB free of FG
                    else:
                        eng.wait_ge(s_of, u)                 # PB free
                    for h in range(2):
                        t = 2 * u + h
                        for c in range(4):
                            sm = s_ra if c % 2 == 0 else s_rv
                            eng.wait_ge(sm, 2 * t + c // 2 + 1)
                            eng.matmul(
                                PB[:, T * h:T * (h + 1)],
                                w2[:, 128 * c:128 * (c + 1)],
                                HR[t % 2][c][:], start=(c == 0), stop=False)
                        # + b2 (outer product with ones row)
                        eng.matmul(PB[:, T * h:T * (h + 1)], b2r[:],
                                   ones[:], start=False, stop=True) \
                           .then_inc(s_op, 1)
                    if u == NP - 1:
                        xpose(u)
            @block.scalar
            def _(eng):
                # const DMAs on the ACT HWDGE queue (x loads run in
                # parallel on the sync queue)
                for src_, dst in ((RED, red), (BCT, bct), (IDB, idb)):
                    eng.dma_start(out=dst[:], in_=src_[:]) \
                       .then_inc(s_cdma1, 16)
                for src_, dst in ((W1BD, w1), (W2BD, w2), (B1BD, b1t),
                                  (B2BD, b2t), (B2R, b2r), (ONESC, ones),
                                  (RNEG2, rn2), (RNEG1, rn1), (CIN, cin)):
                    eng.dma_start(out=dst[:], in_=src_[:]).then_inc(s_cdma, 16)
                # it=0: fixed-node init chain (interleaved with DVE via s_iv)
                eng.wait_ge(s_iv, 1)
                eng.activation(LW[:], W2N[:], AF.Ln).then_inc(s_ln, 1)
                eng.wait_ge(s_iv, 2)
                eng.activation(E1M[:], S0[:], AF.Exp,
                               scale=cin[:, 5:6]).then_inc(s_ln, 1)
                eng.activation(E2M[:], S0[:], AF.Exp,
                               scale=cin[:, 6:7]).then_inc(s_ln, 1)
                eng.wait_ge(s_iv, 3)
                eng.activation(LF[:], FM[:], AF.Ln).then_inc(s_ln, 1)
                # exps for the full iterations + final Ln
                for it in range(NITER_FULL + 1):
                    if it > 0:
                        eng.wait_ge(s_red, NT * (it + 1))
                        eng.activation(LF[:], FG[0:64, :], AF.Ln) \
                           .then_inc(s_ln, 1)
                    if it == NITER_FULL:
                        break
                    for u in range(NP):
                        eng.wait_ge(s_sb, NP * it + u + 1)
                        if u >= 3:
                            eng.wait_ge(s_m, NP * it + u - 2)
                        eng.activation(EM[u % 3][:], SBPR[u % 3], AF.Exp,
                                       scale=rn2[:, 0:1]).then_inc(s_e, 1)
                # final: d, es, oc(u-1), relus(0,1)
                def oc(v):
                    eng.wait_ge(s_pto, v + 1)
                    if v >= 2:
                        eng.wait_ge(s_st2[v % 2], 32 * (v // 2))  # OC free
                    eng.activation(OC[v % 2][:],
                                   TPI[:, 1024 * (v % 2):1024 * (v % 2) + 1024],
                                   AF.Copy).then_inc(s_oc, 1)
                for u in range(NP):
                    eng.wait_ge(s_sb, NP * NITER_FULL + u + 1)
                    if u >= 2:
                        eng.wait_ge(s_xs, u - 1)
                        eng.wait_ge(s_of, u - 1)
                    eng.activation(D[u % 2][:], SBP[0][:], AF.Exp,
                                   scale=rn1[:, 0:1]).then_inc(s_d, 1)
                    eng.activation(ES[u % 2][:], SBP[0][:], AF.Exp,
                                   scale=NU).then_inc(s_des, 1)
                    for h in range(2):
                        t = 2 * u + h
                        for c in (0, 2):
                            eng.wait_ge(s_h, 8 * u + 4 * h + c + 1)
                            if t >= 2:
                                eng.wait_ge(s_op, t - 1)   # HR slot free
                            eng.activation(HR[t % 2][c][:], HP[c % 2], AF.Relu,
                                           bias=b1t[:, c:c + 1]) \
                               .then_inc(s_ra, 1)
                        if h == 0 and u >= 1:
                            # oc fits the ACT bubble between the two
                            # relu groups (h=1's mm1 is still in flight)
                            oc(u - 1)
                oc(NP - 1)

            @block.vector
            def _(eng):
                # XT copies out of TPI + squares, per 8-block group
                for g in range(NG):
                    eng.wait_ge(s_pti, 8 * (g + 1))
                    eng.tensor_copy(XT[:, 1024 * g:1024 * (g + 1)],
                                    TPI[:, 1024 * (g % 2):
                                        1024 * (g % 2) + 1024]) \
                       .then_inc(s_xt, 1)
                    eng.tensor_tensor(A[:, 1024 * g:1024 * (g + 1)],
                                      TPI[:, 1024 * (g % 2):
                                          1024 * (g % 2) + 1024],
                                      XT[:, 1024 * g:1024 * (g + 1)],
                                      ALU.mult).then_inc(s_a, 1)
                # it=0: fixed-node 2-exp init.  F-row = m0, G-row = 2*m1.
                FR = FG[0:64, :]
                GR = FG[64:128, :]
                eng.wait_ge(s_red, NT)
                eng.tensor_scalar(T1[:], FR, cin[:, 0:1], None, ALU.mult)
                eng.tensor_scalar(G2[:], GR, 0.5, None, ALU.mult)
                eng.tensor_tensor(W2N[:], T1[:], G2[:], ALU.subtract)
                eng.tensor_scalar(W2N[:], W2N[:], 1.0, 1e-6, ALU.mult,
                                  ALU.max).then_inc(s_iv, 1)
                # ACT: LW = Ln(W2N)
                eng.wait_ge(s_ln, 1)
                eng.tensor_scalar(S0[:], LW[:], cin[:, 3:4], cin[:, 4:5],
                                  ALU.mult, ALU.add).then_inc(s_iv, 1)
                # ACT: E1M/E2M = Exp(S0 * -2*l)
                eng.tensor_scalar(W2V[:], W2N[:], cin[:, 2:3], None, ALU.mult)
                eng.tensor_tensor(W1V[:], FR, W2V[:], ALU.subtract)
                eng.wait_ge(s_ln, 2)
                eng.tensor_tensor(T1M[:], W1V[:], E1M[:], ALU.mult)
                eng.wait_ge(s_ln, 3)
                eng.tensor_tensor(T2M[:], W2V[:], E2M[:], ALU.mult)
                eng.tensor_tensor(FM[:], T1M[:], T2M[:], ALU.add)
                eng.tensor_scalar(FM[:], FM[:], 1.0, 1e-30, ALU.mult,
                                  ALU.max).then_inc(s_iv, 1)
                # ACT: LF = Ln(FM)
                # G accumulated with -2l weights: RG = -1/G, so the
                # 0.5-scaled Newton step folds into a subtract.
                eng.tensor_scalar(T1[:], T1M[:], cin[:, 5:6], None, ALU.mult)
                eng.tensor_scalar(G2[:], T2M[:], cin[:, 6:7], None, ALU.mult)
                eng.tensor_tensor(GM[:], T1[:], G2[:], ALU.add)  # -2*(G/2)
                eng.reciprocal(RG[:], GM[:])
                eng.wait_ge(s_ln, 4)
                eng.tensor_tensor(P1[:], LF[:], FM[:], ALU.mult)
                eng.tensor_tensor(P1[:], P1[:], RG[:], ALU.mult)
                eng.tensor_tensor(S[:], S0[:], P1[:], ALU.subtract)
                eng.tensor_scalar(SHL[0:64, :], S[:], 1.0, None, ALU.mult)
                eng.tensor_tensor(SHL[64:128, :], S[:], SHL[0:64, :],
                                  ALU.subtract).then_inc(s_sml, 1)
                # m-mults for iteration 0 + the full-iteration chains
                for it in range(NITER_FULL + 1):
                    if it > 0:
                        eng.wait_ge(s_red, NT * (it + 1))
                        eng.reciprocal(RG[:], FG[64:128, :])
                        eng.wait_ge(s_ln, it + 4)
                        eng.tensor_tensor(P1[:], LF[:], FG[0:64, :], ALU.mult)
                        eng.tensor_tensor(P1[:], P1[:], RG[:], ALU.mult)
                        eng.tensor_tensor(S[:], S[:], P1[:], ALU.add)
                        eng.tensor_scalar(SHL[0:64, :], S[:], 1.0, None,
                                          ALU.mult)
                        eng.tensor_tensor(SHL[64:128, :], S[:], SHL[0:64, :],
                                          ALU.subtract).then_inc(s_sml, 1)
                    if it == NITER_FULL:
                        break
                    for u in range(NP):
                        eng.wait_ge(s_e, NP * it + u + 1)
                        if u >= 3:
                            eng.wait_ge(s_red, NT * (it + 1) + 2 * u - 4)
                        eng.tensor_tensor(MM[u % 3][:], EM[u % 3][:],
                                          A[:, 1024 * u:1024 * (u + 1)],
                                          ALU.mult).then_inc(s_m, 1)
                # final: xs, of(u-1), relus(2,3)
                def of(v):
                    eng.wait_ge(s_op, 2 * v + 2)
                    if v >= 2:
                        eng.wait_ge(s_pto, v - 1)   # OF slot free
                    eng.tensor_tensor(OF[v % 2][:], PB[:], ES[v % 2][:],
                                      ALU.mult).then_inc(s_of, 1)
                for u in range(NP):
                    eng.wait_ge(s_d, u + 1)
                    if u >= 2:
                        eng.wait_ge(s_h, 8 * (u - 1))  # XS slot free
                    eng.tensor_tensor(XS[u % 2][:], XT[:, 1024 * u:1024 * (u + 1)],
                                      D[u % 2][:], ALU.mult).then_inc(s_xs, 1)
                    if u >= 1:
                        of(u - 1)
                    for h in range(2):
                        t = 2 * u + h
                        for c in (1, 3):
                            eng.wait_ge(s_h, 8 * u + 4 * h + c + 1)
                            if t >= 2:
                                eng.wait_ge(s_op, t - 1)
                            eng.tensor_scalar(HR[t % 2][c][:], HP[c % 2],
                                              b1t[:, c:c + 1], 0.0,
                                              ALU.add, ALU.max) \
                               .then_inc(s_rv, 1)
                of(NP - 1)

    return nc


_cached = None


def kernel(x, r, W1, b1, W2, b2, _trace=False):
    global _cached, _last_exec_ns
    if _cached is None:
        _cached = _build()
    nc = _cached
    consts = _host_consts(r, W1, b1, W2, b2)
    x = np.ascontiguousarray(np.asarray(x, np.float32))
    in_maps = []
    for i in range(NCORES):
        m = {"x": x[i * BC:(i + 1) * BC]}
        m.update(consts)
        in_maps.append(m)
    res = run_bass_kernel_spmd(nc, in_maps, list(range(NCORES)),
                               trace=_trace)
    _last_exec_ns = res.exec_time_ns
    return np.concatenate([res.results[i]["out"] for i in range(NCORES)],
                          axis=0)

